# revision 27
# baseline (speedup 1.0000x reference)
"""GroupedQueryAttention kernel for 8 Trainium2 NeuronCores.

Sharding: tensor-parallel over KV groups. Core c owns KV group c
(4 query heads x 64 dim): column shards of w_q/w_k/w_v, row shard of
w_o. x is replicated (passed pre-transposed so the contraction dim
lands on SBUF partitions with zero on-device transposes). Each core
computes a partial output x @ .. @ w_o_shard; host sums the partials.

All SBUF tensors are bf16 (PSUM accumulation stays f32); rel-err vs
the f32 reference is ~5e-3, inside the 2e-2 gate.

Structure (per core): packed [wk|wv] M=128 projection, PE-transpose of
V^T into natural layout, Q projection, then a single software-pipelined
stream of 128 attention groups (16 head-chunks x 8 kt-pairs):
  scores S^T of group G+1 (2 matmuls) are emitted BEFORE attn.V of
  group G so ScalarE's exp of G+1 can start while the PE retires G --
  the PE stream is pure back-to-back matmuls paced only by the exp.
Output-projection matmuls of the previous chunk (and the Q projection
of chunk 2 during chunk 0) ride in the stream as fillers, one per
group. Softmax: V1=[V|ones] gives the denominator as O1 row 64;
reciprocal via the fast Newton DVE op; per-head broadcast matmul is
deferred into the next head's groups so the PE never waits on it.
"""

import numpy as np
import ml_dtypes

# ---- problem constants (hardcoded per harness contract) ----
S = 2048          # sequence length
D = 2048          # d_model
N_CORES = 8
HD = 64           # head dim
HPG = 4           # heads per KV group (= per core)
QDIM = HPG * HD   # 256, per-core q width
SCALE = 1.0 / 8.0  # 1/sqrt(HD), exact power of two
SQC = 512         # seq chunk (PSUM bank free size in f32)

_compiled = {}


def build_gqa(s=S, d=D, sqc=SQC, debug=False, debug_taps=False):
    """Build the per-core bass program (SPMD: same program, per-core data)."""
    import concourse.tile as tile
    from concourse import bacc, mybir
    from concourse.masks import make_identity
    from contextlib import ExitStack

    f32 = mybir.dt.float32
    bf16 = mybir.dt.bfloat16
    EXP = mybir.ActivationFunctionType.Exp

    T = s // 128          # seq tiles (sk tiles): 16
    KO = d // 128         # contraction tiles for projections: 16
    QT = QDIM // 128      # q partition tiles (2)
    NCH = s // sqc        # seq chunks: 4
    TPC = sqc // 128      # seq tiles per chunk: 4
    och = 512             # output column chunk width
    NOCH = d // och       # output column chunks: 4
    GPH = T // 2          # groups (kt pairs) per head-chunk: 8

    nc = bacc.Bacc(None, target_bir_lowering=False, debug=debug)
    xT = nc.declare_dram_parameter("xT", [d, s], bf16, isOutput=False)
    wq = nc.declare_dram_parameter("wq", [d, QDIM], bf16, isOutput=False)
    wkv = nc.declare_dram_parameter("wkv", [d, 2 * HD], bf16, isOutput=False)
    wo = nc.declare_dram_parameter("wo", [QDIM, d], bf16, isOutput=False)
    out = nc.declare_dram_parameter("out", [s, d], f32, isOutput=True)
    if debug_taps:
        dbg_qT = nc.declare_dram_parameter("dbg_qT", [128, QT, s], bf16, isOutput=True)
        dbg_kT = nc.declare_dram_parameter("dbg_kT", [128, s], bf16, isOutput=True)
        dbg_v1 = nc.declare_dram_parameter("dbg_v1", [128, T, HD + 1], bf16, isOutput=True)
        dbg_oT = nc.declare_dram_parameter("dbg_oT", [128, QT, s], bf16, isOutput=True)
        dbg_den = nc.declare_dram_parameter("dbg_den", [16, sqc], f32, isOutput=True)
        dbg_rcp = nc.declare_dram_parameter("dbg_rcp", [16, sqc], f32, isOutput=True)

    with tile.TileContext(nc) as tc, ExitStack() as ctx:
        const = ctx.enter_context(tc.tile_pool(name="const", bufs=1))
        persist = ctx.enter_context(tc.tile_pool(name="persist", bufs=1))

        ident = const.tile([128, 128], bf16)
        make_identity(nc, ident)
        ones_row = const.tile([1, HD], f32)
        nc.vector.memset(ones_row, 1.0)
        bias_exp = const.tile([128, 1], f32)
        nc.vector.memset(bias_exp, -8.0)

        kT_sb = persist.tile([128, s], bf16)        # rows 0:64 K^T, 64:128 dup
        v1_sb = persist.tile([128, T, HD + 1], bf16)
        qT_sb = persist.tile([128, QT, s], bf16)
        oT_sb = persist.tile([128, QT, s], bf16)
        wo_sb = persist.tile([128, QT, d], bf16)
        vt_sb = persist.tile([128, s], bf16)        # rows 64:128 hold V^T
        xT_sb = persist.tile([128, KO, s], bf16)
        wq_sb = persist.tile([128, KO, QDIM], bf16)

        nc.vector.memset(v1_sb[:, :, HD:HD + 1], 1.0)

        # ---------------- phase 1: K|V projection + V transpose ----------------
        with (
            tc.tile_pool(name="p1w", bufs=1) as p1w,
            tc.tile_pool(name="p1ps", bufs=6, space="PSUM") as p1ps,
            tc.tile_pool(name="vtps", bufs=2, space="PSUM") as vtps,
        ):
            wkv_sb = p1w.tile([128, KO, 2 * HD], bf16)
            nc.sync.dma_start(
                out=wkv_sb, in_=wkv[:].rearrange("(ko p) m -> p ko m", p=128))
            for ko in range(KO):
                nc.sync.dma_start(
                    out=xT_sb[:, ko, :], in_=xT[ko * 128:(ko + 1) * 128, :])
            nc.sync.dma_start(
                out=wq_sb, in_=wq[:].rearrange("(ko p) m -> p ko m", p=128))
            nc.sync.dma_start(
                out=wo_sb, in_=wo[:].rearrange("(qt p) m -> p qt m", p=128))

            # packed K|V projection for all chunks + Q projection of chunk 0,
            # interleaved ko-outer so the PE tracks the incoming xT slices
            # (the whole block is paced by the xT DMA, not the PE)
            cs0 = slice(0, sqc)
            pkv = [p1ps.tile([128, sqc], f32, name=f"pkv{c}", tag="pp")
                   for c in range(NCH)]
            pq0 = [p1ps.tile([128, sqc], f32, name=f"pq0{qt}", tag="pp")
                   for qt in range(QT)]
            for ko in range(KO):
                for ch in range(NCH):
                    cs = slice(ch * sqc, (ch + 1) * sqc)
                    nc.tensor.matmul(pkv[ch], wkv_sb[:, ko, :], xT_sb[:, ko, cs],
                                     start=(ko == 0), stop=(ko == KO - 1))
                for qt in range(QT):
                    nc.tensor.matmul(
                        pq0[qt], wq_sb[:, ko, qt * 128:(qt + 1) * 128],
                        xT_sb[:, ko, cs0],
                        start=(ko == 0), stop=(ko == KO - 1))
            for ch in range(NCH):
                cs = slice(ch * sqc, (ch + 1) * sqc)
                nc.vector.tensor_copy(out=kT_sb[0:64, cs], in_=pkv[ch][0:64, :])
                nc.vector.tensor_copy(out=vt_sb[64:128, cs], in_=pkv[ch][64:128, :])
            for qt in range(QT):
                nc.vector.tensor_copy(out=qT_sb[:, qt, cs0], in_=pq0[qt])
            nc.sync.dma_start(out=kT_sb[64:128, :], in_=kT_sb[0:64, :])

            # V1 = [V | ones]: PE-transpose V^T tiles into natural layout
            for t in range(T):
                pt = vtps.tile([128, HD], bf16, name="pt")
                nc.tensor.transpose(
                    pt, vt_sb[64:128, t * 128:(t + 1) * 128],
                    ident[64:128, 64:128])
                nc.vector.tensor_copy(out=v1_sb[:, t, 0:HD], in_=pt)

            # Q projection for chunk 1 (chunks 2/3 ride as attention fillers)
            cs1 = slice(sqc, 2 * sqc)
            pq1 = [p1ps.tile([128, sqc], f32, name=f"pq1{qt}", tag="pp")
                   for qt in range(QT)]
            for ko in range(KO):
                for qt in range(QT):
                    nc.tensor.matmul(
                        pq1[qt], wq_sb[:, ko, qt * 128:(qt + 1) * 128],
                        xT_sb[:, ko, cs1],
                        start=(ko == 0), stop=(ko == KO - 1))
            for qt in range(QT):
                nc.vector.tensor_copy(out=qT_sb[:, qt, cs1], in_=pq1[qt])

        # ---------------- phase 2+3: attention + output proj ----------------
        epool = ctx.enter_context(tc.tile_pool(name="epool", bufs=3))
        ev = ctx.enter_context(tc.tile_pool(name="ev", bufs=3))
        ysb = ctx.enter_context(tc.tile_pool(name="ysb", bufs=2))
        scps = ctx.enter_context(tc.tile_pool(name="scps", bufs=2, space="PSUM"))
        otps = ctx.enter_context(tc.tile_pool(name="otps", bufs=2, space="PSUM"))
        bcps = ctx.enter_context(tc.tile_pool(name="bcps", bufs=1, space="PSUM"))
        pyps = ctx.enter_context(tc.tile_pool(name="pyps", bufs=1, space="PSUM"))

        # flat group stream: 16 head-chunks x 8 groups
        HC = [(ch, h) for ch in range(NCH) for h in range(HPG)]

        def sc_tiles():
            # generator of (head-chunk idx, group-in-head, psum tile)
            for i, (ch, h) in enumerate(HC):
                hp = 64 * (h % 2)
                cs = slice(ch * sqc, (ch + 1) * sqc)
                qh = qT_sb[hp:hp + 64, h // 2, cs]
                for g in range(GPH):
                    sc_ps = scps.tile([128, 2, sqc], f32, name="scp")
                    for j in range(2):
                        kt = 2 * g + j
                        nc.tensor.matmul(
                            sc_ps[:, j, :],
                            kT_sb[hp:hp + 64, kt * 128:(kt + 1) * 128],
                            qh, start=True, stop=True)
                    yield (i, g, sc_ps)

        # ---- fillers: queue of pump(pool, tag) closures; each call emits
        # one PE matmul (+ any completion ops) and returns True when done ----
        def outproj_item(t, ns):
            state = {}

            def pump(pool, tag):
                if "py" not in state:
                    state["py"] = pool.tile([128, och], f32, name="pyp", tag=tag)
                    state["qt"] = 0
                qt = state["qt"]
                state["qt"] += 1
                nc.tensor.matmul(
                    state["py"], oT_sb[:, qt, t * 128:(t + 1) * 128],
                    wo_sb[:, qt, ns],
                    start=(qt == 0), stop=(qt == QT - 1))
                if qt == QT - 1:
                    y_sb = ysb.tile([128, och], f32, name="y_sb")
                    nc.vector.tensor_copy(out=y_sb, in_=state["py"])
                    nc.sync.dma_start(
                        out=out[:].rearrange(
                            "(t p) n -> p t n", p=128)[:, t, ns],
                        in_=y_sb)
                return state["qt"] >= QT
            return pump

        def outproj_items(ch):
            return [outproj_item(ch * TPC + tt, slice(n * och, (n + 1) * och))
                    for tt in range(TPC) for n in range(NOCH)]

        def qproj_half(c, qt):
            # Q projection tile (c, qt): 16 accumulating matmuls through the
            # single pyps slot (only ever pumped in-stream)
            cs = slice(c * sqc, (c + 1) * sqc)
            state = {}

            def pump(pool, tag):
                if "pq" not in state:
                    state["pq"] = pyps.tile([128, sqc], f32, name="pyp")
                    state["ko"] = 0
                ko = state["ko"]
                state["ko"] += 1
                nc.tensor.matmul(
                    state["pq"], wq_sb[:, ko, qt * 128:(qt + 1) * 128],
                    xT_sb[:, ko, cs],
                    start=(ko == 0), stop=(ko == KO - 1))
                if ko == KO - 1:
                    nc.vector.tensor_copy(out=qT_sb[:, qt, cs], in_=state["pq"])
                return state["ko"] >= KO
            return pump

        from collections import deque
        fillq = deque()

        def pump_fill(pool, tag):
            if fillq:
                if fillq[0](pool, tag):
                    fillq.popleft()

        # ---- deferred per-head epilogue ----
        def epilogue_dve(i, ot_ps):
            # DVE part: denominator copy + fast reciprocal (emitted at head end)
            den_sb = ev.tile([1, sqc], f32, name="den_sb")
            nc.vector.tensor_copy(out=den_sb, in_=ot_ps[HD:HD + 1, :])
            recip = ev.tile([1, sqc], f32, name="recip")
            with nc.allow_low_precision(reason="~18-bit softmax recip"):
                nc.vector.reciprocal_approx_fast(out=recip, in_=den_sb)
            if debug_taps:
                nc.sync.dma_start(out=dbg_den[i:i + 1, :], in_=den_sb)
                nc.sync.dma_start(out=dbg_rcp[i:i + 1, :], in_=recip)
            return recip

        def epilogue_pe(i, ot_ps, recip):
            # PE broadcast + DVE normalize into oT (emitted a few groups later)
            ch, h = HC[i]
            hp = 64 * (h % 2)
            cs = slice(ch * sqc, (ch + 1) * sqc)
            bc_ps = bcps.tile([HD, sqc], f32, name="bcp")
            nc.tensor.matmul(bc_ps, ones_row, recip, start=True, stop=True)
            onorm = ev.tile([HD, sqc], f32, name="onorm")
            nc.vector.tensor_copy(out=onorm, in_=ot_ps[0:HD, :])
            nc.vector.tensor_mul(
                out=oT_sb[hp:hp + 64, h // 2, cs], in0=onorm, in1=bc_ps)

        # ---- main software-pipelined stream ----
        stream = sc_tiles()
        for c in (2, 3):
            for qt in range(QT):
                fillq.append(qproj_half(c, qt))
        pend = None          # (i, ot_ps, recip) awaiting epilogue_pe
        ot_tiles = {}
        cur = next(stream)
        done = False
        while not done:
            i, g, sc_ps = cur
            # emit next group's scores first (keeps ACT fed one group ahead)
            try:
                nxt = next(stream)
            except StopIteration:
                nxt = None
                done = True
            # filler matmul for this group
            pump_fill(pyps, "pyp")
            # deferred epilogue of the previous head lands early in this head
            if pend is not None and g == 2:
                pi = pend[0]
                epilogue_pe(*pend)
                pend = None
                if HC[pi][1] == HPG - 1:
                    # that chunk's oT is now fully written: its output
                    # projection may enter the filler queue
                    fillq.extend(outproj_items(HC[pi][0]))
            # exp of this group
            e_sb = epool.tile([128, 2, sqc], bf16, name="e_sb")
            nc.scalar.activation(
                out=e_sb, in_=sc_ps, func=EXP, bias=bias_exp, scale=1.0)
            # attn @ V1 accumulation for this group
            if g == 0:
                ot_tiles[i] = otps.tile([128, sqc], f32, name="otp")
            ot_ps = ot_tiles[i]
            for j in range(2):
                kt = 2 * g + j
                nc.tensor.matmul(
                    ot_ps[0:HD + 1, :], v1_sb[:, kt, :], e_sb[:, j, :],
                    start=(kt == 0), stop=(kt == T - 1))
            if g == GPH - 1:
                recip = epilogue_dve(i, ot_ps)
                pend = (i, ot_ps, recip)
                del ot_tiles[i]
            if nxt is not None:
                cur = nxt
        # drain filler overhang first -- keeps the PE busy while the last
        # head's reciprocal chain completes
        while fillq:
            pump_fill(scps, "scp")
        if pend is not None:
            epilogue_pe(*pend)
            pend = None
        # tail: output projection for the last chunk, double-buffered
        # through the (now free) scps pool slots
        fillq.extend(outproj_items(NCH - 1))
        while fillq:
            pump_fill(scps, "scp")

        if debug_taps:
            for (dst, src) in ((dbg_qT, qT_sb), (dbg_kT, kT_sb),
                               (dbg_v1, v1_sb), (dbg_oT, oT_sb)):
                nc.sync.dma_start(out=dst[:], in_=src[:])

    nc.compile()
    return nc


def _get_nc():
    if "nc" not in _compiled:
        _compiled["nc"] = build_gqa()
    return _compiled["nc"]


def _shard_inputs(x, w_q, w_k, w_v, w_o):
    bf16 = ml_dtypes.bfloat16
    x = np.asarray(x, dtype=np.float32)
    w_q = np.asarray(w_q, dtype=np.float32)
    w_k = np.asarray(w_k, dtype=np.float32)
    w_v = np.asarray(w_v, dtype=np.float32)
    w_o = np.asarray(w_o, dtype=np.float32)
    xT = np.ascontiguousarray(x.reshape(S, D).T).astype(bf16)
    in_maps = []
    for c in range(N_CORES):
        wkv = np.concatenate(
            [w_k[:, c * HD:(c + 1) * HD], w_v[:, c * HD:(c + 1) * HD]], axis=1)
        in_maps.append({
            "xT": xT,
            "wq": (w_q[:, c * QDIM:(c + 1) * QDIM] * np.float32(SCALE)).astype(bf16),
            "wkv": np.ascontiguousarray(wkv).astype(bf16),
            "wo": np.ascontiguousarray(w_o[c * QDIM:(c + 1) * QDIM, :]).astype(bf16),
        })
    return in_maps


def kernel(x, w_q, w_k, w_v, w_o):
    from concourse.bass_utils import run_bass_kernel_spmd

    nc = _get_nc()
    in_maps = _shard_inputs(x, w_q, w_k, w_v, w_o)
    res = run_bass_kernel_spmd(nc, in_maps, list(range(N_CORES)))
    acc = np.zeros((S, D), dtype=np.float64)
    for r in res.results:
        acc += r["out"].astype(np.float64)
    return acc.astype(np.float32).reshape(1, S, D)


# revision 32
# speedup vs baseline: 1.0534x; 1.0534x over previous
"""GroupedQueryAttention kernel for 8 Trainium2 NeuronCores.

Sharding: tensor-parallel over KV groups. Core c owns KV group c
(4 query heads x 64 dim): column shards of w_q/w_k/w_v, row shard of
w_o. x is replicated (passed pre-transposed so the contraction dim
lands on SBUF partitions with zero on-device transposes). Each core
computes a partial output x @ .. @ w_o_shard; host sums the partials.

All SBUF tensors are bf16 (PSUM accumulation stays f32); rel-err vs
the f32 reference is ~5e-3, inside the 2e-2 gate.

Structure (per core): packed [wk|wv] M=128 projection, PE-transpose of
V^T into natural layout, Q projection, then a single software-pipelined
stream of 128 attention groups (16 head-chunks x 8 kt-pairs):
  scores S^T of group G+1 (2 matmuls) are emitted BEFORE attn.V of
  group G so ScalarE's exp of G+1 can start while the PE retires G --
  the PE stream is pure back-to-back matmuls paced only by the exp.
Output-projection matmuls of the previous chunk (and the Q projection
of chunk 2 during chunk 0) ride in the stream as fillers, one per
group. Softmax: V1=[V|ones] gives the denominator as O1 row 64;
reciprocal via the fast Newton DVE op; per-head broadcast matmul is
deferred into the next head's groups so the PE never waits on it.
"""

import numpy as np
import ml_dtypes

# ---- problem constants (hardcoded per harness contract) ----
S = 2048          # sequence length
D = 2048          # d_model
N_CORES = 8
HD = 64           # head dim
HPG = 4           # heads per KV group (= per core)
QDIM = HPG * HD   # 256, per-core q width
SCALE = 1.0 / 8.0  # 1/sqrt(HD), exact power of two
SQC = 512         # seq chunk (PSUM bank free size in f32)

_compiled = {}


def build_gqa(s=S, d=D, sqc=SQC, debug=False, debug_taps=False):
    """Build the per-core bass program (SPMD: same program, per-core data)."""
    import concourse.tile as tile
    from concourse import bacc, mybir
    from concourse.masks import make_identity
    from contextlib import ExitStack

    f32 = mybir.dt.float32
    bf16 = mybir.dt.bfloat16
    EXP = mybir.ActivationFunctionType.Exp

    T = s // 128          # seq tiles (sk tiles): 16
    KO = d // 128         # contraction tiles for projections: 16
    QT = QDIM // 128      # q partition tiles (2)
    NCH = s // sqc        # seq chunks: 4
    TPC = sqc // 128      # seq tiles per chunk: 4
    och = 512             # output column chunk width
    NOCH = d // och       # output column chunks: 4
    GPH = T // 2          # groups (kt pairs) per head-chunk: 8

    nc = bacc.Bacc(None, target_bir_lowering=False, debug=debug)
    xT = nc.declare_dram_parameter("xT", [d, s], bf16, isOutput=False)
    wq = nc.declare_dram_parameter("wq", [d, QDIM], bf16, isOutput=False)
    wkv = nc.declare_dram_parameter("wkv", [d, 2 * HD], bf16, isOutput=False)
    wo = nc.declare_dram_parameter("wo", [QDIM, d], bf16, isOutput=False)
    out = nc.declare_dram_parameter("out", [s, d], bf16, isOutput=True)
    if debug_taps:
        dbg_qT = nc.declare_dram_parameter("dbg_qT", [128, QT, s], bf16, isOutput=True)
        dbg_kT = nc.declare_dram_parameter("dbg_kT", [128, s], bf16, isOutput=True)
        dbg_v1 = nc.declare_dram_parameter("dbg_v1", [128, T, HD + 1], bf16, isOutput=True)
        dbg_oT = nc.declare_dram_parameter("dbg_oT", [128, QT, s], bf16, isOutput=True)
        dbg_den = nc.declare_dram_parameter("dbg_den", [16, sqc], f32, isOutput=True)
        dbg_rcp = nc.declare_dram_parameter("dbg_rcp", [16, sqc], f32, isOutput=True)

    with tile.TileContext(nc) as tc, ExitStack() as ctx:
        const = ctx.enter_context(tc.tile_pool(name="const", bufs=1))
        persist = ctx.enter_context(tc.tile_pool(name="persist", bufs=1))

        ident = const.tile([128, 128], bf16)
        make_identity(nc, ident)
        ones_row = const.tile([1, HD], f32)
        nc.vector.memset(ones_row, 1.0)
        bias_exp = const.tile([128, 1], f32)
        nc.vector.memset(bias_exp, -8.0)

        kT_sb = persist.tile([128, s], bf16)        # rows 0:64 K^T, 64:128 dup
        v1_sb = persist.tile([128, T, HD + 1], bf16)
        qT_sb = persist.tile([128, QT, s], bf16)
        oT_sb = persist.tile([128, QT, s], bf16)
        wo_sb = persist.tile([128, QT, d], bf16)
        vt_sb = persist.tile([128, s], bf16)        # rows 64:128 hold V^T
        xT_sb = persist.tile([128, KO, s], bf16)
        wq_sb = persist.tile([128, KO, QDIM], bf16)

        nc.vector.memset(v1_sb[:, :, HD:HD + 1], 1.0)

        # ---------------- phase 1: K|V projection + V transpose ----------------
        with (
            tc.tile_pool(name="p1w", bufs=1) as p1w,
            tc.tile_pool(name="p1ps", bufs=6, space="PSUM") as p1ps,
            tc.tile_pool(name="vtps", bufs=2, space="PSUM") as vtps,
        ):
            wkv_sb = p1w.tile([128, KO, 2 * HD], bf16)
            nc.sync.dma_start(
                out=wkv_sb, in_=wkv[:].rearrange("(ko p) m -> p ko m", p=128))
            for ko in range(KO):
                nc.sync.dma_start(
                    out=xT_sb[:, ko, :], in_=xT[ko * 128:(ko + 1) * 128, :])
            nc.sync.dma_start(
                out=wq_sb, in_=wq[:].rearrange("(ko p) m -> p ko m", p=128))
            nc.sync.dma_start(
                out=wo_sb, in_=wo[:].rearrange("(qt p) m -> p qt m", p=128))

            # packed K|V projection for all chunks + Q projection of chunk 0,
            # interleaved ko-outer so the PE tracks the incoming xT slices
            # (the whole block is paced by the xT DMA, not the PE)
            cs0 = slice(0, sqc)
            pkv = [p1ps.tile([128, sqc], f32, name=f"pkv{c}", tag="pp")
                   for c in range(NCH)]
            pq0 = [p1ps.tile([128, sqc], f32, name=f"pq0{qt}", tag="pp")
                   for qt in range(QT)]
            for ko in range(KO):
                for ch in range(NCH):
                    cs = slice(ch * sqc, (ch + 1) * sqc)
                    nc.tensor.matmul(pkv[ch], wkv_sb[:, ko, :], xT_sb[:, ko, cs],
                                     start=(ko == 0), stop=(ko == KO - 1))
                for qt in range(QT):
                    nc.tensor.matmul(
                        pq0[qt], wq_sb[:, ko, qt * 128:(qt + 1) * 128],
                        xT_sb[:, ko, cs0],
                        start=(ko == 0), stop=(ko == KO - 1))
            for ch in range(NCH):
                cs = slice(ch * sqc, (ch + 1) * sqc)
                nc.vector.tensor_copy(out=kT_sb[0:64, cs], in_=pkv[ch][0:64, :])
                nc.vector.tensor_copy(out=vt_sb[64:128, cs], in_=pkv[ch][64:128, :])
            for qt in range(QT):
                nc.vector.tensor_copy(out=qT_sb[:, qt, cs0], in_=pq0[qt])
            nc.sync.dma_start(out=kT_sb[64:128, :], in_=kT_sb[0:64, :])

            # V1 = [V | ones]: PE-transpose V^T tiles into natural layout
            for t in range(T):
                pt = vtps.tile([128, HD], bf16, name="pt")
                nc.tensor.transpose(
                    pt, vt_sb[64:128, t * 128:(t + 1) * 128],
                    ident[64:128, 64:128])
                nc.vector.tensor_copy(out=v1_sb[:, t, 0:HD], in_=pt)

            # Q projection for chunk 1 (chunks 2/3 ride as attention fillers)
            cs1 = slice(sqc, 2 * sqc)
            pq1 = [p1ps.tile([128, sqc], f32, name=f"pq1{qt}", tag="pp")
                   for qt in range(QT)]
            for ko in range(KO):
                for qt in range(QT):
                    nc.tensor.matmul(
                        pq1[qt], wq_sb[:, ko, qt * 128:(qt + 1) * 128],
                        xT_sb[:, ko, cs1],
                        start=(ko == 0), stop=(ko == KO - 1))
            for qt in range(QT):
                nc.vector.tensor_copy(out=qT_sb[:, qt, cs1], in_=pq1[qt])

        # ---------------- phase 2+3: attention + output proj ----------------
        epool = ctx.enter_context(tc.tile_pool(name="epool", bufs=3))
        ev = ctx.enter_context(tc.tile_pool(name="ev", bufs=3))
        ysb = ctx.enter_context(tc.tile_pool(name="ysb", bufs=2))
        scps = ctx.enter_context(tc.tile_pool(name="scps", bufs=2, space="PSUM"))
        otps = ctx.enter_context(tc.tile_pool(name="otps", bufs=2, space="PSUM"))
        bcps = ctx.enter_context(tc.tile_pool(name="bcps", bufs=1, space="PSUM"))
        pyps = ctx.enter_context(tc.tile_pool(name="pyps", bufs=1, space="PSUM"))

        # flat group stream: 16 head-chunks x 8 groups
        HC = [(ch, h) for ch in range(NCH) for h in range(HPG)]

        def sc_tiles():
            # generator of (head-chunk idx, group-in-head, psum tile)
            for i, (ch, h) in enumerate(HC):
                hp = 64 * (h % 2)
                cs = slice(ch * sqc, (ch + 1) * sqc)
                qh = qT_sb[hp:hp + 64, h // 2, cs]
                for g in range(GPH):
                    sc_ps = scps.tile([128, 2, sqc], f32, name="scp")
                    for j in range(2):
                        kt = 2 * g + j
                        nc.tensor.matmul(
                            sc_ps[:, j, :],
                            kT_sb[hp:hp + 64, kt * 128:(kt + 1) * 128],
                            qh, start=True, stop=True)
                    yield (i, g, sc_ps)

        # ---- fillers: queue of pump(pool, tag) closures; each call emits
        # one PE matmul (+ any completion ops) and returns True when done ----
        def outproj_item(t, ns, act_evac=False):
            state = {}

            def pump(pool, tag):
                if "py" not in state:
                    state["py"] = pool.tile([128, och], f32, name="pyp", tag=tag)
                    state["qt"] = 0
                qt = state["qt"]
                state["qt"] += 1
                nc.tensor.matmul(
                    state["py"], oT_sb[:, qt, t * 128:(t + 1) * 128],
                    wo_sb[:, qt, ns],
                    start=(qt == 0), stop=(qt == QT - 1))
                if qt == QT - 1:
                    y_sb = ysb.tile([128, och], bf16, name="y_sb")
                    if act_evac:
                        nc.scalar.copy(out=y_sb, in_=state["py"])
                    else:
                        nc.vector.tensor_copy(out=y_sb, in_=state["py"])
                    nc.sync.dma_start(
                        out=out[:].rearrange(
                            "(t p) n -> p t n", p=128)[:, t, ns],
                        in_=y_sb)
                return state["qt"] >= QT
            return pump

        def outproj_items(ch):
            return [outproj_item(ch * TPC + tt, slice(n * och, (n + 1) * och))
                    for tt in range(TPC) for n in range(NOCH)]

        def qproj_half(c, qt):
            # Q projection tile (c, qt): 16 accumulating matmuls through the
            # single pyps slot (only ever pumped in-stream)
            cs = slice(c * sqc, (c + 1) * sqc)
            state = {}

            def pump(pool, tag):
                if "pq" not in state:
                    state["pq"] = pyps.tile([128, sqc], f32, name="pyp")
                    state["ko"] = 0
                ko = state["ko"]
                state["ko"] += 1
                nc.tensor.matmul(
                    state["pq"], wq_sb[:, ko, qt * 128:(qt + 1) * 128],
                    xT_sb[:, ko, cs],
                    start=(ko == 0), stop=(ko == KO - 1))
                if ko == KO - 1:
                    nc.vector.tensor_copy(out=qT_sb[:, qt, cs], in_=state["pq"])
                return state["ko"] >= KO
            return pump

        from collections import deque
        fillq = deque()

        def pump_fill(pool, tag):
            if fillq:
                if fillq[0](pool, tag):
                    fillq.popleft()

        # ---- deferred per-head epilogue ----
        def epilogue_dve(i, ot_ps):
            # DVE part: denominator copy + fast reciprocal (emitted at head end)
            den_sb = ev.tile([1, sqc], f32, name="den_sb")
            nc.vector.tensor_copy(out=den_sb, in_=ot_ps[HD:HD + 1, :])
            recip = ev.tile([1, sqc], f32, name="recip")
            with nc.allow_low_precision(reason="~18-bit softmax recip"):
                nc.vector.reciprocal_approx_fast(out=recip, in_=den_sb)
            if debug_taps:
                nc.sync.dma_start(out=dbg_den[i:i + 1, :], in_=den_sb)
                nc.sync.dma_start(out=dbg_rcp[i:i + 1, :], in_=recip)
            return recip

        def epilogue_pe(i, ot_ps, recip):
            # PE broadcast + DVE normalize into oT (emitted a few groups later)
            ch, h = HC[i]
            hp = 64 * (h % 2)
            cs = slice(ch * sqc, (ch + 1) * sqc)
            bc_ps = bcps.tile([HD, sqc], f32, name="bcp")
            nc.tensor.matmul(bc_ps, ones_row, recip, start=True, stop=True)
            onorm = ev.tile([HD, sqc], f32, name="onorm")
            nc.vector.tensor_copy(out=onorm, in_=ot_ps[0:HD, :])
            nc.vector.tensor_mul(
                out=oT_sb[hp:hp + 64, h // 2, cs], in0=onorm, in1=bc_ps)

        # ---- main software-pipelined stream ----
        stream = sc_tiles()
        for c in (2, 3):
            for qt in range(QT):
                fillq.append(qproj_half(c, qt))
        pend = None          # (i, ot_ps, recip) awaiting epilogue_pe
        ot_tiles = {}
        cur = next(stream)
        done = False
        while not done:
            i, g, sc_ps = cur
            # emit next group's scores first (keeps ACT fed one group ahead)
            try:
                nxt = next(stream)
            except StopIteration:
                nxt = None
                done = True
            # filler matmul for this group
            pump_fill(pyps, "pyp")
            # deferred epilogue of the previous head lands early in this head
            if pend is not None and g == 2:
                pi = pend[0]
                epilogue_pe(*pend)
                pend = None
                if HC[pi][1] == HPG - 1:
                    # that chunk's oT is now fully written: its output
                    # projection may enter the filler queue
                    fillq.extend(outproj_items(HC[pi][0]))
            # exp of this group
            e_sb = epool.tile([128, 2, sqc], bf16, name="e_sb")
            nc.scalar.activation(
                out=e_sb, in_=sc_ps, func=EXP, bias=bias_exp, scale=1.0)
            # attn @ V1 accumulation for this group
            if g == 0:
                ot_tiles[i] = otps.tile([128, sqc], f32, name="otp")
            ot_ps = ot_tiles[i]
            for j in range(2):
                kt = 2 * g + j
                nc.tensor.matmul(
                    ot_ps[0:HD + 1, :], v1_sb[:, kt, :], e_sb[:, j, :],
                    start=(kt == 0), stop=(kt == T - 1))
            if g == GPH - 1:
                recip = epilogue_dve(i, ot_ps)
                pend = (i, ot_ps, recip)
                del ot_tiles[i]
            if nxt is not None:
                cur = nxt
        # tail: drain the filler overhang, then the last chunk's output
        # projection. Alternate PSUM pools and evacuation engines between
        # consecutive items so they pipeline instead of serializing on one
        # slot (the overhang drains first, keeping the PE busy while the
        # last head's reciprocal chain completes).
        def drain_tail(items, k0=0):
            k = k0
            for item in items:
                pool, tag = (scps, "scp") if k % 2 == 0 else (otps, "otp")
                while not item(pool, tag):
                    pass
                k += 1
            return k

        # last head's epilogue must be emitted before any tail item can
        # re-claim its otps slot (slot reuse races with later-emitted reads)
        if pend is not None:
            epilogue_pe(*pend)
            pend = None
        overhang = []
        while fillq:
            overhang.append(fillq.popleft())
        k = drain_tail(overhang)
        tail_items = [outproj_item(
            (NCH - 1) * TPC + tt, slice(n * och, (n + 1) * och),
            act_evac=((tt * NOCH + n) % 2 == 1))
            for tt in range(TPC) for n in range(NOCH)]
        drain_tail(tail_items, k0=k)

        if debug_taps:
            for (dst, src) in ((dbg_qT, qT_sb), (dbg_kT, kT_sb),
                               (dbg_v1, v1_sb), (dbg_oT, oT_sb)):
                nc.sync.dma_start(out=dst[:], in_=src[:])

    nc.compile()
    return nc


def _get_nc():
    if "nc" not in _compiled:
        _compiled["nc"] = build_gqa()
    return _compiled["nc"]


def _shard_inputs(x, w_q, w_k, w_v, w_o):
    bf16 = ml_dtypes.bfloat16
    x = np.asarray(x, dtype=np.float32)
    w_q = np.asarray(w_q, dtype=np.float32)
    w_k = np.asarray(w_k, dtype=np.float32)
    w_v = np.asarray(w_v, dtype=np.float32)
    w_o = np.asarray(w_o, dtype=np.float32)
    xT = np.ascontiguousarray(x.reshape(S, D).T).astype(bf16)
    in_maps = []
    for c in range(N_CORES):
        wkv = np.concatenate(
            [w_k[:, c * HD:(c + 1) * HD], w_v[:, c * HD:(c + 1) * HD]], axis=1)
        in_maps.append({
            "xT": xT,
            "wq": (w_q[:, c * QDIM:(c + 1) * QDIM] * np.float32(SCALE)).astype(bf16),
            "wkv": np.ascontiguousarray(wkv).astype(bf16),
            "wo": np.ascontiguousarray(w_o[c * QDIM:(c + 1) * QDIM, :]).astype(bf16),
        })
    return in_maps


def kernel(x, w_q, w_k, w_v, w_o):
    from concourse.bass_utils import run_bass_kernel_spmd

    nc = _get_nc()
    in_maps = _shard_inputs(x, w_q, w_k, w_v, w_o)
    res = run_bass_kernel_spmd(nc, in_maps, list(range(N_CORES)))
    acc = np.zeros((S, D), dtype=np.float64)
    for r in res.results:
        acc += r["out"].astype(np.float64)
    return acc.astype(np.float32).reshape(1, S, D)


# revision 33
# speedup vs baseline: 1.1184x; 1.0617x over previous
"""GroupedQueryAttention kernel for 8 Trainium2 NeuronCores.

Sharding: tensor-parallel over KV groups. Core c owns KV group c
(4 query heads x 64 dim): column shards of w_q/w_k/w_v, row shard of
w_o. x is replicated (passed pre-transposed so the contraction dim
lands on SBUF partitions with zero on-device transposes). Each core
computes a partial output x @ .. @ w_o_shard; host sums the partials.

All SBUF tensors are bf16 (PSUM accumulation stays f32); rel-err vs
the f32 reference is ~5e-3, inside the 2e-2 gate.

Structure (per core): packed [wk|wv] M=128 projection, PE-transpose of
V^T into natural layout, Q projection, then a single software-pipelined
stream of 128 attention groups (16 head-chunks x 8 kt-pairs):
  scores S^T of group G+1 (2 matmuls) are emitted BEFORE attn.V of
  group G so ScalarE's exp of G+1 can start while the PE retires G --
  the PE stream is pure back-to-back matmuls paced only by the exp.
Output-projection matmuls of the previous chunk (and the Q projection
of chunk 2 during chunk 0) ride in the stream as fillers, one per
group. Softmax: V1=[V|ones] gives the denominator as O1 row 64;
reciprocal via the fast Newton DVE op; per-head broadcast matmul is
deferred into the next head's groups so the PE never waits on it.
"""

import numpy as np
import ml_dtypes

# ---- problem constants (hardcoded per harness contract) ----
S = 2048          # sequence length
D = 2048          # d_model
N_CORES = 8
HD = 64           # head dim
HPG = 4           # heads per KV group (= per core)
QDIM = HPG * HD   # 256, per-core q width
SCALE = 1.0 / 8.0  # 1/sqrt(HD), exact power of two
SQC = 512         # seq chunk (PSUM bank free size in f32)

_compiled = {}


def build_gqa(s=S, d=D, sqc=SQC, debug=False, debug_taps=False):
    """Build the per-core bass program (SPMD: same program, per-core data)."""
    import concourse.tile as tile
    from concourse import bacc, mybir
    from concourse.masks import make_identity
    from contextlib import ExitStack

    f32 = mybir.dt.float32
    bf16 = mybir.dt.bfloat16
    EXP = mybir.ActivationFunctionType.Exp

    T = s // 128          # seq tiles (sk tiles): 16
    KO = d // 128         # contraction tiles for projections: 16
    QT = QDIM // 128      # q partition tiles (2)
    NCH = s // sqc        # seq chunks: 4
    TPC = sqc // 128      # seq tiles per chunk: 4
    och = 512             # output column chunk width
    NOCH = d // och       # output column chunks: 4
    GPH = T // 2          # groups (kt pairs) per head-chunk: 8

    nc = bacc.Bacc(None, target_bir_lowering=False, debug=debug)
    xT = nc.declare_dram_parameter("xT", [d, s], bf16, isOutput=False)
    wq = nc.declare_dram_parameter("wq", [d, QDIM], bf16, isOutput=False)
    wkv = nc.declare_dram_parameter("wkv", [d, 2 * HD], bf16, isOutput=False)
    wo = nc.declare_dram_parameter("wo", [QDIM, d], bf16, isOutput=False)
    out = nc.declare_dram_parameter("out", [s, d], bf16, isOutput=True)
    if debug_taps:
        dbg_qT = nc.declare_dram_parameter("dbg_qT", [128, QT, s], bf16, isOutput=True)
        dbg_kT = nc.declare_dram_parameter("dbg_kT", [128, s], bf16, isOutput=True)
        dbg_v1 = nc.declare_dram_parameter("dbg_v1", [128, T, HD + 1], bf16, isOutput=True)
        dbg_oT = nc.declare_dram_parameter("dbg_oT", [128, QT, s], bf16, isOutput=True)
        dbg_den = nc.declare_dram_parameter("dbg_den", [16, sqc], f32, isOutput=True)
        dbg_rcp = nc.declare_dram_parameter("dbg_rcp", [16, sqc], f32, isOutput=True)

    with tile.TileContext(nc) as tc, ExitStack() as ctx:
        const = ctx.enter_context(tc.tile_pool(name="const", bufs=1))
        persist = ctx.enter_context(tc.tile_pool(name="persist", bufs=1))

        ident = const.tile([128, 128], bf16)
        make_identity(nc, ident)
        ones_row = const.tile([1, HD], f32)
        nc.vector.memset(ones_row, 1.0)
        bias_exp = const.tile([128, 1], f32)
        nc.vector.memset(bias_exp, -8.0)

        kT_sb = persist.tile([128, s], bf16)        # rows 0:64 K^T, 64:128 dup
        v1_sb = persist.tile([128, T, HD + 1], bf16)
        qT_sb = persist.tile([128, QT, s], bf16)
        oT_sb = persist.tile([128, QT, s], bf16)
        wo_sb = persist.tile([128, QT, d], bf16)
        vt_sb = persist.tile([128, s], bf16)        # rows 64:128 hold V^T
        xT_sb = persist.tile([128, KO, s], bf16)
        wq_sb = persist.tile([128, KO, QDIM], bf16)

        nc.vector.memset(v1_sb[:, :, HD:HD + 1], 1.0)

        # ---------------- phase 1: K|V projection + V transpose ----------------
        with (
            tc.tile_pool(name="p1w", bufs=1) as p1w,
            tc.tile_pool(name="p1ps", bufs=6, space="PSUM") as p1ps,
            tc.tile_pool(name="vtps", bufs=2, space="PSUM") as vtps,
        ):
            wkv_sb = p1w.tile([128, KO, 2 * HD], bf16)
            nc.sync.dma_start(
                out=wkv_sb, in_=wkv[:].rearrange("(ko p) m -> p ko m", p=128))
            for ko in range(KO):
                nc.sync.dma_start(
                    out=xT_sb[:, ko, :], in_=xT[ko * 128:(ko + 1) * 128, :])
            nc.sync.dma_start(
                out=wq_sb, in_=wq[:].rearrange("(ko p) m -> p ko m", p=128))
            nc.sync.dma_start(
                out=wo_sb, in_=wo[:].rearrange("(qt p) m -> p qt m", p=128))

            # packed K|V projection for all chunks + Q projection of chunk 0,
            # interleaved ko-outer so the PE tracks the incoming xT slices
            # (the whole block is paced by the xT DMA, not the PE)
            cs0 = slice(0, sqc)
            pkv = [p1ps.tile([128, sqc], f32, name=f"pkv{c}", tag="pp")
                   for c in range(NCH)]
            pq0 = [p1ps.tile([128, sqc], f32, name=f"pq0{qt}", tag="pp")
                   for qt in range(QT)]
            for ko in range(KO):
                for ch in range(NCH):
                    cs = slice(ch * sqc, (ch + 1) * sqc)
                    nc.tensor.matmul(pkv[ch], wkv_sb[:, ko, :], xT_sb[:, ko, cs],
                                     start=(ko == 0), stop=(ko == KO - 1))
                for qt in range(QT):
                    nc.tensor.matmul(
                        pq0[qt], wq_sb[:, ko, qt * 128:(qt + 1) * 128],
                        xT_sb[:, ko, cs0],
                        start=(ko == 0), stop=(ko == KO - 1))
            for ch in range(NCH):
                cs = slice(ch * sqc, (ch + 1) * sqc)
                nc.vector.tensor_copy(out=kT_sb[0:64, cs], in_=pkv[ch][0:64, :])
                nc.vector.tensor_copy(out=vt_sb[64:128, cs], in_=pkv[ch][64:128, :])
            for qt in range(QT):
                nc.vector.tensor_copy(out=qT_sb[:, qt, cs0], in_=pq0[qt])
            nc.sync.dma_start(out=kT_sb[64:128, :], in_=kT_sb[0:64, :])

            # V1 = [V | ones]: PE-transpose V^T tiles into natural layout
            for t in range(T):
                pt = vtps.tile([128, HD], bf16, name="pt")
                nc.tensor.transpose(
                    pt, vt_sb[64:128, t * 128:(t + 1) * 128],
                    ident[64:128, 64:128])
                nc.vector.tensor_copy(out=v1_sb[:, t, 0:HD], in_=pt)

            # Q projection for chunk 1 (chunks 2/3 ride as attention fillers)
            cs1 = slice(sqc, 2 * sqc)
            pq1 = [p1ps.tile([128, sqc], f32, name=f"pq1{qt}", tag="pp")
                   for qt in range(QT)]
            for ko in range(KO):
                for qt in range(QT):
                    nc.tensor.matmul(
                        pq1[qt], wq_sb[:, ko, qt * 128:(qt + 1) * 128],
                        xT_sb[:, ko, cs1],
                        start=(ko == 0), stop=(ko == KO - 1))
            for qt in range(QT):
                nc.vector.tensor_copy(out=qT_sb[:, qt, cs1], in_=pq1[qt])

        # ---------------- phase 2+3: attention + output proj ----------------
        epool = ctx.enter_context(tc.tile_pool(name="epool", bufs=3))
        ev = ctx.enter_context(tc.tile_pool(name="ev", bufs=3))
        ysb = ctx.enter_context(tc.tile_pool(name="ysb", bufs=2))
        scps = ctx.enter_context(tc.tile_pool(name="scps", bufs=2, space="PSUM"))
        otps = ctx.enter_context(tc.tile_pool(name="otps", bufs=2, space="PSUM"))
        bcps = ctx.enter_context(tc.tile_pool(name="bcps", bufs=1, space="PSUM"))
        pyps = ctx.enter_context(tc.tile_pool(name="pyps", bufs=1, space="PSUM"))

        # flat group stream: 16 head-chunks x 8 groups
        HC = [(ch, h) for ch in range(NCH) for h in range(HPG)]

        def sc_tiles():
            # generator of (head-chunk idx, group-in-head, psum tile)
            for i, (ch, h) in enumerate(HC):
                hp = 64 * (h % 2)
                cs = slice(ch * sqc, (ch + 1) * sqc)
                qh = qT_sb[hp:hp + 64, h // 2, cs]
                for g in range(GPH):
                    sc_ps = scps.tile([128, 2, sqc], f32, name="scp")
                    for j in range(2):
                        kt = 2 * g + j
                        nc.tensor.matmul(
                            sc_ps[:, j, :],
                            kT_sb[hp:hp + 64, kt * 128:(kt + 1) * 128],
                            qh, start=True, stop=True)
                    yield (i, g, sc_ps)

        # ---- fillers: queue of pump(pool, tag) closures; each call emits
        # one PE matmul (+ any completion ops) and returns True when done ----
        def outproj_item(t, ns, act_evac=False):
            state = {}

            def pump(pool, tag):
                if "py" not in state:
                    state["py"] = pool.tile([128, och], f32, name="pyp", tag=tag)
                    state["qt"] = 0
                qt = state["qt"]
                state["qt"] += 1
                nc.tensor.matmul(
                    state["py"], oT_sb[:, qt, t * 128:(t + 1) * 128],
                    wo_sb[:, qt, ns],
                    start=(qt == 0), stop=(qt == QT - 1))
                if qt == QT - 1:
                    y_sb = ysb.tile([128, och], bf16, name="y_sb")
                    if act_evac:
                        nc.scalar.copy(out=y_sb, in_=state["py"])
                    else:
                        nc.vector.tensor_copy(out=y_sb, in_=state["py"])
                    nc.sync.dma_start(
                        out=out[:].rearrange(
                            "(t p) n -> p t n", p=128)[:, t, ns],
                        in_=y_sb)
                return state["qt"] >= QT
            return pump

        def outproj_items(ch):
            return [outproj_item(ch * TPC + tt, slice(n * och, (n + 1) * och))
                    for tt in range(TPC) for n in range(NOCH)]

        def qproj_half(c, qt):
            # Q projection tile (c, qt): 16 accumulating matmuls through the
            # single pyps slot (only ever pumped in-stream)
            cs = slice(c * sqc, (c + 1) * sqc)
            state = {}

            def pump(pool, tag):
                if "pq" not in state:
                    state["pq"] = pyps.tile([128, sqc], f32, name="pyp")
                    state["ko"] = 0
                ko = state["ko"]
                state["ko"] += 1
                nc.tensor.matmul(
                    state["pq"], wq_sb[:, ko, qt * 128:(qt + 1) * 128],
                    xT_sb[:, ko, cs],
                    start=(ko == 0), stop=(ko == KO - 1))
                if ko == KO - 1:
                    nc.vector.tensor_copy(out=qT_sb[:, qt, cs], in_=state["pq"])
                return state["ko"] >= KO
            return pump

        from collections import deque
        fillq = deque()

        def pump_fill(pool, tag):
            if fillq:
                if fillq[0](pool, tag):
                    fillq.popleft()

        # ---- deferred per-head epilogue ----
        def epilogue_dve(i, ot_ps):
            # DVE part: denominator copy + fast reciprocal (emitted at head end)
            den_sb = ev.tile([1, sqc], f32, name="den_sb")
            nc.vector.tensor_copy(out=den_sb, in_=ot_ps[HD:HD + 1, :])
            recip = ev.tile([1, sqc], f32, name="recip")
            with nc.allow_low_precision(reason="~18-bit softmax recip"):
                nc.vector.reciprocal_approx_fast(out=recip, in_=den_sb)
            if debug_taps:
                nc.sync.dma_start(out=dbg_den[i:i + 1, :], in_=den_sb)
                nc.sync.dma_start(out=dbg_rcp[i:i + 1, :], in_=recip)
            return recip

        def epilogue_pe(i, ot_ps, recip):
            # PE broadcast + DVE normalize into oT (emitted a few groups later)
            ch, h = HC[i]
            hp = 64 * (h % 2)
            cs = slice(ch * sqc, (ch + 1) * sqc)
            bc_ps = bcps.tile([HD, sqc], f32, name="bcp")
            nc.tensor.matmul(bc_ps, ones_row, recip, start=True, stop=True)
            onorm = ev.tile([HD, sqc], f32, name="onorm")
            nc.vector.tensor_copy(out=onorm, in_=ot_ps[0:HD, :])
            nc.vector.tensor_mul(
                out=oT_sb[hp:hp + 64, h // 2, cs], in0=onorm, in1=bc_ps)

        # ---- main software-pipelined stream ----
        # PE queue per iteration G: [sc(G+1) pair | filler(s) | ot(G-1) pair]
        # -- the attn.V matmuls trail the exp by a full group, so the PE
        # never waits on the exp it just enabled; ScalarE alone sets the pace.
        ot_tiles = {}
        pend_box = [None]    # (i, ot_ps, recip) awaiting epilogue_pe

        def emit_ot(i, g, e_sb):
            if g == 0:
                ot_tiles[i] = otps.tile([128, sqc], f32, name="otp")
            ot_ps = ot_tiles[i]
            for j in range(2):
                kt = 2 * g + j
                nc.tensor.matmul(
                    ot_ps[0:HD + 1, :], v1_sb[:, kt, :], e_sb[:, j, :],
                    start=(kt == 0), stop=(kt == T - 1))
            if g == GPH - 1:
                recip = epilogue_dve(i, ot_ps)
                pend_box[0] = (i, ot_ps, recip)
                del ot_tiles[i]

        stream = sc_tiles()
        for c in (2, 3):
            for qt in range(QT):
                fillq.append(qproj_half(c, qt))
        prev = None          # (i, g, e_sb) whose ot emission is delayed
        cur = next(stream)
        done = False
        while not done:
            i, g, sc_ps = cur
            # emit next group's scores first (keeps ACT fed one group ahead)
            try:
                nxt = next(stream)
            except StopIteration:
                nxt = None
                done = True
            # filler matmul(s) for this group; drain backlog at two per slot
            pump_fill(pyps, "pyp")
            if len(fillq) >= 2:
                pump_fill(pyps, "pyp")
            # deferred epilogue of the previous head, far enough in that the
            # reciprocal chain is guaranteed complete (no PE wait)
            if pend_box[0] is not None and g == 4:
                pi = pend_box[0][0]
                epilogue_pe(*pend_box[0])
                pend_box[0] = None
                if HC[pi][1] == HPG - 1:
                    # that chunk's oT is now fully written: its output
                    # projection may enter the filler queue
                    fillq.extend(outproj_items(HC[pi][0]))
            # exp of this group
            e_sb = epool.tile([128, 2, sqc], bf16, name="e_sb")
            nc.scalar.activation(
                out=e_sb, in_=sc_ps, func=EXP, bias=bias_exp, scale=1.0)
            # attn @ V1 accumulation, one group behind
            if prev is not None:
                emit_ot(*prev)
            prev = (i, g, e_sb)
            if nxt is not None:
                cur = nxt
        emit_ot(*prev)
        pend = pend_box[0]
        # tail: drain the filler overhang, then the last chunk's output
        # projection. Alternate PSUM pools and evacuation engines between
        # consecutive items so they pipeline instead of serializing on one
        # slot (the overhang drains first, keeping the PE busy while the
        # last head's reciprocal chain completes).
        def drain_tail(items, k0=0):
            k = k0
            for item in items:
                pool, tag = (scps, "scp") if k % 2 == 0 else (otps, "otp")
                while not item(pool, tag):
                    pass
                k += 1
            return k

        # last head's epilogue must be emitted before any tail item can
        # re-claim its otps slot (slot reuse races with later-emitted reads)
        if pend is not None:
            epilogue_pe(*pend)
            pend = None
        overhang = []
        while fillq:
            overhang.append(fillq.popleft())
        k = drain_tail(overhang)
        tail_items = [outproj_item(
            (NCH - 1) * TPC + tt, slice(n * och, (n + 1) * och),
            act_evac=((tt * NOCH + n) % 2 == 1))
            for tt in range(TPC) for n in range(NOCH)]
        drain_tail(tail_items, k0=k)

        if debug_taps:
            for (dst, src) in ((dbg_qT, qT_sb), (dbg_kT, kT_sb),
                               (dbg_v1, v1_sb), (dbg_oT, oT_sb)):
                nc.sync.dma_start(out=dst[:], in_=src[:])

    nc.compile()
    return nc


def _get_nc():
    if "nc" not in _compiled:
        _compiled["nc"] = build_gqa()
    return _compiled["nc"]


def _shard_inputs(x, w_q, w_k, w_v, w_o):
    bf16 = ml_dtypes.bfloat16
    x = np.asarray(x, dtype=np.float32)
    w_q = np.asarray(w_q, dtype=np.float32)
    w_k = np.asarray(w_k, dtype=np.float32)
    w_v = np.asarray(w_v, dtype=np.float32)
    w_o = np.asarray(w_o, dtype=np.float32)
    xT = np.ascontiguousarray(x.reshape(S, D).T).astype(bf16)
    in_maps = []
    for c in range(N_CORES):
        wkv = np.concatenate(
            [w_k[:, c * HD:(c + 1) * HD], w_v[:, c * HD:(c + 1) * HD]], axis=1)
        in_maps.append({
            "xT": xT,
            "wq": (w_q[:, c * QDIM:(c + 1) * QDIM] * np.float32(SCALE)).astype(bf16),
            "wkv": np.ascontiguousarray(wkv).astype(bf16),
            "wo": np.ascontiguousarray(w_o[c * QDIM:(c + 1) * QDIM, :]).astype(bf16),
        })
    return in_maps


def kernel(x, w_q, w_k, w_v, w_o):
    from concourse.bass_utils import run_bass_kernel_spmd

    nc = _get_nc()
    in_maps = _shard_inputs(x, w_q, w_k, w_v, w_o)
    res = run_bass_kernel_spmd(nc, in_maps, list(range(N_CORES)))
    acc = np.zeros((S, D), dtype=np.float64)
    for r in res.results:
        acc += r["out"].astype(np.float64)
    return acc.astype(np.float32).reshape(1, S, D)
